# revision 1
# baseline (speedup 1.0000x reference)
"""Cross-attention Trainium2 Bass kernel (nn_CrossAttention, B=4, Sq=Skv=2048,
query_dim=1024, kv_dim=768, H=16, D=64) on 8 NeuronCores.

Sharding: core c -> (batch b = c//2, head-group g = c%2 of 8 heads = 512 dims).
Each core computes its head-group's Q/K/V projections, attention, and a
partial output projection (ctx_g @ Wo_g). Host sums the two partials per
batch and adds the bias terms (bo + bv @ Wo, exact because softmax rows sum
to 1, so the V-bias contributes bv @ Wo to every row).

Device layout tricks:
  - All activations enter transposed (host transposes): qT/kT/vT [dim, seq].
  - Q/K projections produce QT/KT in [head-dim, seq] "pair layout": dd-tile
    t (128 partitions) = heads 2t (partitions 0:64) and 2t+1 (64:128).
  - Scores are computed transposed: S^T[j, q] = KT_h^T @ QT_h (contraction
    over the 64 head dims on partitions), so softmax's j axis lands on
    partitions and E=exp(S^T) is directly the moving operand of the ctx
    matmul with lhsT = V_h (natural [j, d] layout, no transposes).
  - V is augmented with a ones column (65th) -> ctx matmul row 64 yields
    the softmax denominators for free.
  - Normalization: sums staged on partition 64, repacked by DMA to [8, q],
    reciprocal on 8 lanes, broadcast back to 64 partitions by step-0-free-dim
    DMA, multiplied into ctx during the f32r cast.
  - All matmul operands are float32r (TF32-like, 1 cyc/row at N>=512 vs 4
    for fp32); producers write f32r natively (walrus requires it).
"""

import sys

sys.path.insert(0, "/opt/trn_rl_repo")

import numpy as np

import concourse.bass as bass  # noqa: F401
import concourse.tile as tile
from concourse import bacc, mybir
from concourse.bass_utils import run_bass_kernel_spmd

F32 = mybir.dt.float32
F32R = mybir.dt.float32r
EXP = mybir.ActivationFunctionType.Exp

QDIM = 1024
KVDIM = 768
H_CORE = 8  # heads per core
D = 64
GDIM = H_CORE * D  # 512, head-group dims per core
KQ = QDIM // 128  # 8  k-chunks for Q proj
KKV = KVDIM // 128  # 6  k-chunks for K/V proj
NB = 512  # q-block size
VCOL = D + 1  # 65, V columns incl. ones


def build_program(sq: int, skv: int):
    """Build the per-core Bass program. Returns (nc, names)."""
    nc = bacc.Bacc("TRN2", target_bir_lowering=False, debug=False)

    qt_d = nc.dram_tensor("qT", [KQ, 128, sq], F32R, kind="ExternalInput")
    kt_d = nc.dram_tensor("kT", [KKV, 128, skv], F32R, kind="ExternalInput")
    vt_d = nc.dram_tensor("vT", [KKV, 128, skv], F32R, kind="ExternalInput")
    wq_d = nc.dram_tensor("wq", [KQ, 128, GDIM], F32R, kind="ExternalInput")
    wk_d = nc.dram_tensor("wk", [KKV, 128, GDIM], F32R, kind="ExternalInput")
    wv_d = nc.dram_tensor("wv", [KKV, 128, GDIM], F32R, kind="ExternalInput")
    wo_d = nc.dram_tensor("wo", [4, 128, QDIM], F32R, kind="ExternalInput")
    bq_d = nc.dram_tensor("bq", [4, 128], F32, kind="ExternalInput")
    bk_d = nc.dram_tensor("bk", [4, 128], F32, kind="ExternalInput")
    out_d = nc.dram_tensor("out", [sq, QDIM], F32, kind="ExternalOutput")

    n_qb = sq // NB  # q blocks
    n_jc = skv // 128  # kv chunks (j tiles)
    n_jo = skv // 128  # V_sb j-outer count
    s_scale = 1.0 / np.sqrt(D)

    with tile.TileContext(nc) as tc:
        with (
            tc.tile_pool(name="sb", bufs=1) as sb,
            tc.tile_pool(name="ps", bufs=1, space="PSUM") as ps,
        ):
            # ---- resident weights ----
            wq_sb = sb.tile([128, KQ, GDIM], F32R, tag="wq")
            nc.sync.dma_start(wq_sb, wq_d.ap().rearrange("k p n -> p k n"))
            wk_sb = sb.tile([128, KKV, GDIM], F32R, tag="wk")
            nc.sync.dma_start(wk_sb, wk_d.ap().rearrange("k p n -> p k n"))
            wv_sb = sb.tile([128, KKV, GDIM], F32R, tag="wv")
            nc.sync.dma_start(wv_sb, wv_d.ap().rearrange("k p n -> p k n"))
            wo_sb = sb.tile([128, 4, QDIM], F32R, tag="wo")
            nc.sync.dma_start(wo_sb, wo_d.ap().rearrange("k p n -> p k n"))
            bq_sb = sb.tile([128, 4], F32, tag="bq")
            nc.sync.dma_start(bq_sb, bq_d.ap().rearrange("t p -> p t"))
            bk_sb = sb.tile([128, 4], F32, tag="bk")
            nc.sync.dma_start(bk_sb, bk_d.ap().rearrange("t p -> p t"))
            ones_f32 = sb.tile([128, 1], F32, tag="ones")
            nc.vector.memset(ones_f32, 1.0)

            # ---- resident K^T (pair layout) and V (+ones) ----
            kt_sb = sb.tile([128, 4, skv], F32R, tag="ktr")
            v_sb = sb.tile([128, n_jo, H_CORE * VCOL], F32R, tag="vsb")
            for jo in range(n_jo):
                nc.vector.tensor_copy(
                    v_sb[:, jo, :].rearrange("p (h d) -> p h d", d=VCOL)[:, :, D : D + 1],
                    ones_f32[:, 0:1].to_broadcast((128, H_CORE, 1)),
                )

            # K projection: KT[dd, j] = Wk_g^T @ kT
            for q4 in range(skv // 512):
                kps = [
                    ps.tile([128, 512], F32, tag="mm", bufs=4, name=f"kps{t}")
                    for t in range(4)
                ]
                for kc in range(KKV):
                    ktc = sb.tile([128, 512], F32R, tag="chunk", bufs=2, name="ktc")
                    nc.sync.dma_start(ktc, kt_d.ap()[kc, :, q4 * 512 : (q4 + 1) * 512])
                    for t in range(4):
                        nc.tensor.matmul(
                            kps[t],
                            wk_sb[:, kc, t * 128 : (t + 1) * 128],
                            ktc,
                            start=(kc == 0),
                            stop=(kc == KKV - 1),
                        )
                for t in range(4):
                    nc.vector.tensor_scalar_add(
                        out=kt_sb[:, t, q4 * 512 : (q4 + 1) * 512],
                        in0=kps[t],
                        scalar1=bk_sb[:, t : t + 1],
                    )

            # V projection: V[j, dd] = vT_chunk^T @ Wv_g
            for q4 in range(skv // 512):
                vps = [
                    ps.tile([128, 512], F32, tag="mm", bufs=4, name=f"vps{t}")
                    for t in range(4)
                ]
                for kc in range(KKV):
                    vtc = sb.tile([128, 512], F32R, tag="chunk", bufs=2, name="vtc")
                    nc.sync.dma_start(vtc, vt_d.ap()[kc, :, q4 * 512 : (q4 + 1) * 512])
                    for t in range(4):
                        nc.tensor.matmul(
                            vps[t],
                            vtc[:, t * 128 : (t + 1) * 128],
                            wv_sb[:, kc, :],
                            start=(kc == 0),
                            stop=(kc == KKV - 1),
                        )
                for t in range(4):
                    jo = q4 * 4 + t
                    nc.vector.tensor_copy(
                        v_sb[:, jo, :].rearrange("p (h d) -> p h d", d=VCOL)[
                            :, :, 0:D
                        ],
                        vps[t].rearrange("p (h d) -> p h d", d=D),
                    )

            # ---- per q-block: Q proj, attention, out proj ----
            for qb in range(n_qb):
                qsl = slice(qb * NB, (qb + 1) * NB)

                qt_blk = sb.tile([128, 4, NB], F32R, tag="qt", name="qt_blk")
                qps = [
                    ps.tile([128, 512], F32, tag="mm", bufs=4, name=f"qps{t}")
                    for t in range(4)
                ]
                for kc in range(KQ):
                    qtc = sb.tile([128, NB], F32R, tag="chunk", bufs=2, name="qtc")
                    nc.sync.dma_start(qtc, qt_d.ap()[kc, :, qsl])
                    for t in range(4):
                        nc.tensor.matmul(
                            qps[t],
                            wq_sb[:, kc, t * 128 : (t + 1) * 128],
                            qtc,
                            start=(kc == 0),
                            stop=(kc == KQ - 1),
                        )
                for t in range(4):
                    nc.vector.tensor_scalar_add(
                        out=qt_blk[:, t, :], in0=qps[t], scalar1=bq_sb[:, t : t + 1]
                    )

                stage = sb.tile([128, H_CORE, NB], F32, tag="stage", name="stage")
                ctxu = sb.tile([128, 4, NB], F32, tag="ctxu", name="ctxu")

                for h in range(H_CORE):
                    pair, sub = h // 2, h % 2
                    psl = slice(sub * 64, sub * 64 + 64)
                    ctx_ps = ps.tile([128, NB], F32, tag="ctx", bufs=2, name="ctx_ps")
                    for jc in range(n_jc):
                        st_ps = ps.tile([128, NB], F32, tag="st", bufs=2, name="st_ps")
                        nc.tensor.matmul(
                            st_ps,
                            kt_sb[psl, pair, jc * 128 : (jc + 1) * 128],
                            qt_blk[psl, pair, :],
                            start=True,
                            stop=True,
                            skip_group_check=True,
                        )
                        e_t = sb.tile([128, NB], F32R, tag="e", bufs=3, name="e_t")
                        nc.scalar.activation(out=e_t, in_=st_ps, func=EXP, scale=s_scale)
                        nc.tensor.matmul(
                            ctx_ps[0:VCOL, :],
                            v_sb[:, jc, h * VCOL : (h + 1) * VCOL],
                            e_t,
                            start=(jc == 0),
                            stop=(jc == n_jc - 1),
                            skip_group_check=True,
                        )
                    # stage sums on partition 64; ctx into pair layout
                    nc.vector.tensor_copy(stage[64:65, h, :], ctx_ps[64:65, :])
                    nc.vector.tensor_copy(ctxu[psl, pair, :], ctx_ps[0:64, :])

                # denominators: repack -> reciprocal -> broadcast -> multiply
                sums_sb = sb.tile([H_CORE, NB], F32, tag="sums", bufs=2, name="sums_sb")
                nc.sync.dma_start(sums_sb, stage[64:65, :, :])
                rsum_sb = sb.tile([H_CORE, NB], F32, tag="rsum", bufs=2, name="rsum_sb")
                nc.vector.reciprocal(out=rsum_sb, in_=sums_sb)

                ctxn = sb.tile([128, 4, NB], F32R, tag="ctxn", name="ctxn")
                for pair in range(4):
                    rb = sb.tile([128, NB], F32, tag="rb", bufs=2, name="rb")
                    for sub in range(2):
                        h = pair * 2 + sub
                        nc.sync.dma_start(
                            rb[sub * 64 : sub * 64 + 64, :],
                            rsum_sb[h : h + 1, None, :].to_broadcast((1, 64, NB)),
                        )
                    nc.vector.tensor_mul(
                        out=ctxn[:, pair, :], in0=ctxu[:, pair, :], in1=rb
                    )

                # out projection: out[s, n] = ctxn^T @ Wo_g  (partial)
                for sti in range(NB // 128):
                    osb = sb.tile([128, QDIM], F32, tag="osb", bufs=2, name="osb")
                    for nh in range(2):
                        ops = ps.tile([128, 512], F32, tag="mm", bufs=4, name="ops")
                        for c in range(4):
                            nc.tensor.matmul(
                                ops,
                                ctxn[:, c, sti * 128 : (sti + 1) * 128],
                                wo_sb[:, c, nh * 512 : (nh + 1) * 512],
                                start=(c == 0),
                                stop=(c == 3),
                            )
                        nc.vector.tensor_copy(osb[:, nh * 512 : (nh + 1) * 512], ops)
                    r0 = qb * NB + sti * 128
                    nc.sync.dma_start(out_d.ap()[r0 : r0 + 128, :], osb)

    nc.compile()
    return nc


_NC_CACHE = {}


def _get_nc(sq, skv):
    key = (sq, skv)
    if key not in _NC_CACHE:
        _NC_CACHE[key] = build_program(sq, skv)
    return _NC_CACHE[key]


def make_in_maps(query, key, value, Wq, bq, Wk, bk, Wv, bv, Wo, bo):
    B = query.shape[0]
    f = np.float32
    per_batch = []
    for b in range(B):
        per_batch.append(
            (
                np.ascontiguousarray(query[b].T, f).reshape(KQ, 128, -1),
                np.ascontiguousarray(key[b].T, f).reshape(KKV, 128, -1),
                np.ascontiguousarray(value[b].T, f).reshape(KKV, 128, -1),
            )
        )
    per_group = []
    for g in range(2):
        gs = slice(g * GDIM, (g + 1) * GDIM)
        per_group.append(
            dict(
                wq=np.ascontiguousarray(Wq[:, gs], f).reshape(KQ, 128, GDIM),
                wk=np.ascontiguousarray(Wk[:, gs], f).reshape(KKV, 128, GDIM),
                wv=np.ascontiguousarray(Wv[:, gs], f).reshape(KKV, 128, GDIM),
                wo=np.ascontiguousarray(Wo[gs, :], f).reshape(4, 128, QDIM),
                bq=np.ascontiguousarray(bq[gs], f).reshape(4, 128),
                bk=np.ascontiguousarray(bk[gs], f).reshape(4, 128),
            )
        )
    in_maps = []
    for c in range(2 * B):
        b, g = c // 2, c % 2
        qT, kT, vT = per_batch[b]
        m = dict(qT=qT, kT=kT, vT=vT)
        m.update(per_group[g])
        in_maps.append(m)
    return in_maps


def kernel(query, key, value, Wq, bq, Wk, bk, Wv, bv, Wo, bo, _trace=False):
    B, sq, _ = query.shape
    skv = key.shape[1]
    nc = _get_nc(sq, skv)
    in_maps = make_in_maps(query, key, value, Wq, bq, Wk, bk, Wv, bv, Wo, bo)
    res = run_bass_kernel_spmd(
        nc, in_maps, core_ids=list(range(len(in_maps))), trace=_trace
    )
    bias_eff = (
        bo.astype(np.float64) + bv.astype(np.float64) @ Wo.astype(np.float64)
    ).astype(np.float32)
    out = np.empty((B, sq, QDIM), np.float32)
    for b in range(B):
        out[b] = res.results[2 * b]["out"] + res.results[2 * b + 1]["out"] + bias_eff
    if _trace:
        return out, res
    return out
